# revision 23
# baseline (speedup 1.0000x reference)
"""Trainium2 Bass kernel for AceStep sliding-window GQA attention.

Problem: B=2, S=2048, H=2048, 16 Q heads / 4 KV heads, D=128, window +-256, fp32.

Sharding: 8 cores = (batch b in {0,1}) x (kv-group g in {0..3}).
Each core owns 4 Q heads + 1 KV head and computes a partial output
(wo restricted to its head group); host sums 4 partials per batch.

On-device layout is fully transposed ([dim, token]) so that:
  - QKV projections:  qT[d,s] = wqT[H,d].T @ hsT[H,s]          (PE matmul)
  - RoPE rotate_half: rot(q) = R @ q  (128x128 rotation matrix) (PE matmul)
  - RMSNorm sum over d and softmax denominator sum over k
    (partition-axis reductions) via ones-vector matmuls          (PE matmul)
  - scoresT[k,q] = kT[d,k].T @ qT[d,q]                          (PE matmul)
  - PV: outT[d,q] = v_kd[k,d].T @ probsT[k,q]                   (PE matmul)
  - O-proj: finalT[ho,s] = woT[dq,ho].T @ attnT[dq,s]           (PE matmul)
Softmax is computed without max-subtraction: RMS-normed q,k bound
|score| <= sqrt(128) ~ 11.3, so exp stays in fp32 range.
Sliding window exploited at block level: only ~6 of 16 k-tiles per q-tile.
Matmuls run as float32r (full PE rate at N>=256, near-fp32 precision).
"""

import os
import sys
from contextlib import ExitStack

import numpy as np

for _p in ("/opt/trn_rl_repo", "/root/.axon_site/_ro/trn_rl_repo"):
    if os.path.isdir(_p) and _p not in sys.path:
        sys.path.insert(0, _p)

import concourse.bass as bass
import concourse.bacc as bacc
import concourse.mybir as mybir
from concourse import tile
from concourse.alu_op_type import AluOpType
from concourse import bass_isa

F32 = mybir.dt.float32
F32R = mybir.dt.float32r
BF16 = mybir.dt.bfloat16
CDT = BF16 if os.environ.get("BASS_CDT", "f32r") == "bf16" else F32R
MDT = BF16 if CDT == BF16 else F32   # mask dtype
ACT = mybir.ActivationFunctionType

# problem dims (hardcoded per spec)
B, S, H, NHQ, NKV, D, WIN = 2, 2048, 2048, 16, 4, 128, 256
EPS = 1e-6
HPC = NHQ // NKV          # 4 q heads per core
DQ = HPC * D              # 512
P = 128
KT = H // P               # 16 contraction tiles
SQ = 512                  # s-quarter width for projections
NSQ = S // SQ
QTW = 256                 # attention q-tile width
NQT = S // QTW
NKTILES = (QTW + 2 * WIN) // P   # 6 k-tiles per q-tile
N_CORES = 8

_CACHE = {}


def build_nc():
    nc = bacc.Bacc(None, target_bir_lowering=False, debug=False)

    hsT = nc.dram_tensor("hsT", [H, S], CDT, kind="ExternalInput")
    wq_t = nc.dram_tensor("wq_t", [H, DQ], CDT, kind="ExternalInput")
    wk_t = nc.dram_tensor("wk_t", [H, D], CDT, kind="ExternalInput")
    wv_t = nc.dram_tensor("wv_t", [H, D], CDT, kind="ExternalInput")
    wo_t = nc.dram_tensor("wo_t", [DQ, H], CDT, kind="ExternalInput")
    cos_t = nc.dram_tensor("cos_t", [D, S], F32, kind="ExternalInput")
    sin2_t = nc.dram_tensor("sin2_t", [D, S], F32, kind="ExternalInput")
    rot_t = nc.dram_tensor("rot_t", [D, D], CDT, kind="ExternalInput")
    ones_d = nc.dram_tensor("ones_d", [P, P], CDT, kind="ExternalInput")
    maskb_d = nc.dram_tensor("maskb", [4, P, 2 * QTW], CDT, kind="ExternalInput")
    ident_d = nc.dram_tensor("ident_d", [P, P], CDT, kind="ExternalInput")
    outT = nc.dram_tensor("outT", [H, S], F32, kind="ExternalOutput")

    with tile.TileContext(nc) as tc:
        es = ExitStack()
        top = es.enter_context(tc.tile_pool(name="top", bufs=1))

        # const APs used by nc.scalar.activation float biases
        zc = top.tile([P, 1], F32)
        nc.vector.memset(zc[:, :], 0.0)
        nc.const_aps.aps[(F32, 0.0)] = zc[:, :]
        bq = top.tile([P, 1], F32)
        nc.vector.memset(bq[:, :], float(D * EPS))
        nc.const_aps.aps[(F32, float(D * EPS))] = bq[:, :]
        bk = top.tile([P, 1], F32)
        nc.vector.memset(bk[:, :], float(EPS))
        nc.const_aps.aps[(F32, float(EPS))] = bk[:, :]

        ident = top.tile([P, P], F32)
        nc.vector.memset(ident[:, :], 1.0)
        nc.gpsimd.affine_select(
            out=ident[:, :], in_=ident[:, :], pattern=[[-1, P]],
            compare_op=AluOpType.is_equal, fill=0.0, base=0, channel_multiplier=1,
        )
        rot_sb = top.tile([D, D], CDT)
        nc.sync.dma_start(out=rot_sb[:, :], in_=rot_t[:, :])
        ones_t = top.tile([P, P], CDT)
        nc.sync.dma_start(out=ones_t[:, :], in_=ones_d[:, :])

        qTall = top.tile([P, HPC * S], CDT, name="qTall")  # col = qi*1024 + h*256 + q
        kTt = top.tile([P, S], CDT, name="kTt")
        vkd = top.tile([P, S], CDT, name="vkd")  # s-tile t at [:, t*P:(t+1)*P], [s,d] layout
        attnT = [top.tile([P, S], CDT, name=f"attnT{h}") for h in range(HPC)]

        # ---------------- Phase 1: QKV projections + RMSNorm + RoPE ----------
        with tc.tile_pool(name="ph1", bufs=1) as ph1, \
             tc.tile_pool(name="ph1p", bufs=1, space="PSUM") as ph1p:
            wq_sb = ph1.tile([P, KT * DQ], CDT)
            wk_sb = ph1.tile([P, KT * D], CDT)
            wv_sb = ph1.tile([P, KT * D], CDT)
            cos_sb = ph1.tile([D, S], F32)
            sin2_sb = ph1.tile([D, S], F32)

            def load_weights_k(k):
                nc.sync.dma_start(out=wq_sb[:, k * DQ:(k + 1) * DQ], in_=wq_t[k * P:(k + 1) * P, :])
                nc.sync.dma_start(out=wk_sb[:, k * D:(k + 1) * D], in_=wk_t[k * P:(k + 1) * P, :])
                nc.sync.dma_start(out=wv_sb[:, k * D:(k + 1) * D], in_=wv_t[k * P:(k + 1) * P, :])

            for sq in range(NSQ):
                s0 = sq * SQ
                hst = []
                for k in range(KT):
                    if sq == 0:
                        load_weights_k(k)  # interleave so matmuls start early
                    t = ph1.tile([P, SQ], CDT, tag="hst", bufs=8)
                    nc.scalar.dma_start(out=t[:, :], in_=hsT[k * P:(k + 1) * P, s0:s0 + SQ])
                    hst.append(t)
                if sq == 0:
                    nc.sync.dma_start(out=cos_sb[:, :], in_=cos_t[:, :])
                    nc.sync.dma_start(out=sin2_sb[:, :], in_=sin2_t[:, :])

                accs = [ph1p.tile([P, SQ], F32, tag=f"acc{m}", bufs=1, name=f"acc{m}_{sq}")
                        for m in range(HPC + 2)]
                for k in range(KT):
                    st, sp = (k == 0), (k == KT - 1)
                    for m in range(HPC):
                        nc.tensor.matmul(
                            accs[m][:, :],
                            wq_sb[:, k * DQ + m * D: k * DQ + (m + 1) * D],
                            hst[k][:, :], start=st, stop=sp)
                    nc.tensor.matmul(accs[HPC][:, :], wk_sb[:, k * D:(k + 1) * D],
                                     hst[k][:, :], start=st, stop=sp)
                    nc.tensor.matmul(accs[HPC + 1][:, :], wv_sb[:, k * D:(k + 1) * D],
                                     hst[k][:, :], start=st, stop=sp)

                # v first: releases its accumulator early
                vsb = ph1.tile([P, SQ], F32, tag="tmp", bufs=6, name=f"vsb{sq}")
                nc.scalar.copy(vsb[:, :], accs[HPC + 1][:, :])
                for j in range(SQ // P):
                    vt = ph1p.tile([P, P], F32, tag="rot", bufs=2, name=f"vt{sq}_{j}")
                    nc.tensor.transpose(vt[:, :], vsb[:, j * P:(j + 1) * P], ident[:, :])
                    nc.vector.tensor_copy(vkd[:, s0 + j * P: s0 + (j + 1) * P], vt[:, :])

                # q heads + k: RMSNorm (scale folded for q) + RoPE.
                # Stage-batched per engine so the slow gpsimd partition reduces
                # never head-of-line-block the ACT/DVE FIFOs that release the
                # PSUM accumulators for the next quarter.
                sqts, t1s, t2s, rots, ssqBs, rmsBs, invBs_l = {}, {}, {}, {}, {}, {}, {}
                for m in range(HPC + 1):
                    raw = accs[m]
                    sqt = ph1.tile([P, SQ], CDT, tag="sqt", bufs=4, name=f"sqt{sq}_{m}")
                    nc.scalar.activation(sqt[:, :], raw[:, :], ACT.Square)
                    sqts[m] = sqt
                    t1 = ph1.tile([P, SQ], CDT, tag="t1", bufs=4, name=f"t1_{sq}_{m}")
                    nc.vector.tensor_mul(t1[:, :], raw[:, :], sin2_sb[:, s0:s0 + SQ])
                    t1s[m] = t1
                    t2 = ph1.tile([P, SQ], F32, tag="tmp", bufs=6, name=f"t2_{sq}_{m}")
                    nc.vector.tensor_mul(t2[:, :], raw[:, :], cos_sb[:, s0:s0 + SQ])
                    t2s[m] = t2
                for m in range(HPC + 1):
                    rotp = ph1p.tile([P, SQ], F32, tag="rot", bufs=2, name=f"rot{sq}_{m}")
                    nc.tensor.matmul(rotp[:, :], rot_sb[:, :], t1s[m][:, :])
                    rots[m] = rotp
                    ssqB = ph1.tile([P, SQ], F32, tag="ssqB", bufs=2, name=f"ssqB{sq}_{m}")
                    nc.gpsimd.partition_all_reduce(ssqB[:, :], sqts[m][:, :], channels=P,
                                                   reduce_op=bass_isa.ReduceOp.add)
                    ssqBs[m] = ssqB
                for m in range(HPC + 1):
                    rmsB = ph1.tile([P, SQ], F32, tag="rmsB", bufs=2, name=f"rmsB{sq}_{m}")
                    if m < HPC:
                        nc.scalar.activation(rmsB[:, :], ssqBs[m][:, :], ACT.Sqrt,
                                             bias=float(D * EPS), scale=1.0)
                    else:
                        nc.scalar.activation(rmsB[:, :], ssqBs[m][:, :], ACT.Sqrt,
                                             bias=float(EPS), scale=1.0 / D)
                    rmsBs[m] = rmsB
                    # rope combine can run as soon as rotp lands
                    t3 = ph1.tile([P, SQ], F32, tag="t3", bufs=3, name=f"t3_{sq}_{m}")
                    nc.vector.tensor_add(t3[:, :], t2s[m][:, :], rots[m][:, :])
                    t2s[m] = t3
                for m in range(HPC + 1):
                    invB = ph1.tile([P, SQ], F32, tag="invBs", bufs=2, name=f"invB{sq}_{m}")
                    nc.vector.reciprocal_approx_fast(out=invB[:, :], in_=rmsBs[m][:, :])
                    if m < HPC:
                        # [128, 2, 256] strided view: quarter sq covers qi = 2sq, 2sq+1
                        dst = qTall[:, :].rearrange(
                            "p (qi h q) -> p qi h q", h=HPC, q=QTW)[:, 2 * sq:2 * sq + 2, m, :]
                        nc.vector.tensor_mul(dst, t2s[m][:, :].rearrange("p (a q) -> p a q", a=2),
                                             invB[:, :].rearrange("p (a q) -> p a q", a=2))
                    else:
                        nc.vector.tensor_mul(kTt[:, s0:s0 + SQ], t2s[m][:, :], invB[:, :])

        # ---------------- Phase 2+3 -----------------------------------------
        with tc.tile_pool(name="late", bufs=1) as late:
            wo_sb = late.tile([P, HPC * H], CDT)
            for dqt in range(HPC):
                nc.sync.dma_start(out=wo_sb[:, dqt * H:(dqt + 1) * H],
                                  in_=wo_t[dqt * P:(dqt + 1) * P, :])

            # ------- attention (qi-outer, heads inner) + interleaved O-proj ----
            with tc.tile_pool(name="att", bufs=1) as att, \
                 tc.tile_pool(name="attp", bufs=1, space="PSUM") as attp:
                maskb_sb = att.tile([P, 4 * 2 * QTW], CDT)
                for t in range(4):
                    nc.sync.dma_start(out=maskb_sb[:, t * 2 * QTW:(t + 1) * 2 * QTW], in_=maskb_d[t])
                identr_sb = att.tile([P, P], CDT)
                nc.sync.dma_start(out=identr_sb[:, :], in_=ident_d[:, :])

                def attend_pair(hp, qi):
                    q0 = qi * QTW
                    col0 = qi * (HPC * QTW) + hp * (2 * QTW)
                    W2 = 2 * QTW
                    tl = [t for t in range(NKTILES) if 0 <= q0 - WIN + t * P <= S - P]
                    L = len(tl)
                    probs_all = att.tile([P, NKTILES * W2], CDT, tag="probs", bufs=3,
                                         name=f"probs{hp}_{qi}")
                    BIDX = {0: 0, 1: 1, 4: 2, 5: 3}
                    for t in tl:
                        ks = q0 - WIN + t * P
                        scp = attp.tile([P, W2], F32, tag="sc", bufs=2, name=f"sc{hp}_{qi}_{t}")
                        masked = t in BIDX
                        nc.tensor.matmul(scp[:, :], kTt[:, ks:ks + P],
                                         qTall[:, col0:col0 + W2],
                                         start=True, stop=not masked)
                        if masked:  # window mask as additive bias accumulated on PE
                            bi = BIDX[t]
                            nc.tensor.matmul(scp[:, :], identr_sb[:, :],
                                             maskb_sb[:, bi * W2:(bi + 1) * W2],
                                             start=False, stop=True)
                        nc.scalar.activation(probs_all[:, t * W2:(t + 1) * W2],
                                             scp[:, :], ACT.Exp)
                    den = attp.tile([1, W2], F32, tag="den", bufs=1, name=f"den{hp}_{qi}")
                    pvs2 = []
                    for h2 in range(2):
                        pv = attp.tile([P, QTW], F32, tag="pv", bufs=2, name=f"pv{hp}_{qi}_{h2}")
                        for i, t in enumerate(tl):
                            ks = q0 - WIN + t * P
                            nc.tensor.matmul(pv[:, :], vkd[:, ks:ks + P],
                                             probs_all[:, t * W2 + h2 * QTW: t * W2 + (h2 + 1) * QTW],
                                             start=(i == 0), stop=(i == L - 1))
                        pvs = att.tile([P, QTW], F32, tag="pvs", bufs=4, name=f"pvs{hp}_{qi}_{h2}")
                        nc.vector.tensor_copy(pvs[:, :], pv[:, :])
                        pvs2.append(pvs)
                    for i, t in enumerate(tl):
                        nc.tensor.matmul(den[:, :], ones_t[:, 0:1],
                                         probs_all[:, t * W2:(t + 1) * W2],
                                         start=(i == 0), stop=(i == L - 1))
                    invf = att.tile([1, W2], F32, tag="invf", bufs=3, name=f"invf{hp}_{qi}")
                    nc.vector.reciprocal_approx_fast(out=invf[:, :], in_=den[:, :])
                    invr = att.tile([1, W2], CDT, tag="invr", bufs=3, name=f"invr{hp}_{qi}")
                    nc.scalar.copy(invr[:, :], invf[:, :])
                    invB = attp.tile([P, W2], F32, tag="ainvB", bufs=1, name=f"ainvB{hp}_{qi}")
                    nc.tensor.matmul(invB[:, :], ones_t[0:1, :], invr[:, :])
                    for h2 in range(2):
                        nc.vector.tensor_mul(attnT[2 * hp + h2][:, q0:q0 + QTW],
                                             pvs2[h2][:, :],
                                             invB[:, h2 * QTW:(h2 + 1) * QTW])

                def oproj_block(st4):
                    s0 = st4 * SQ
                    for ho in range(H // P):
                        ops = attp.tile([P, SQ], F32, tag="o", bufs=2, name=f"o{st4}_{ho}")
                        for dqt in range(HPC):
                            nc.tensor.matmul(
                                ops[:, :],
                                wo_sb[:, dqt * H + ho * P: dqt * H + (ho + 1) * P],
                                attnT[dqt][:, s0:s0 + SQ],
                                start=(dqt == 0), stop=(dqt == HPC - 1))
                        ob = att.tile([P, SQ], F32, tag="ob", bufs=4, name=f"ob{st4}_{ho}")
                        if (ho + st4) % 2 == 0:
                            nc.scalar.copy(ob[:, :], ops[:, :])
                        else:
                            nc.vector.tensor_copy(ob[:, :], ops[:, :])
                        nc.sync.dma_start(out=outT[ho * P:(ho + 1) * P, s0:s0 + SQ], in_=ob[:, :])

                for qi in range(NQT):
                    for hp in range(HPC // 2):
                        attend_pair(hp, qi)
                    if qi % 2 == 1:
                        oproj_block(qi // 2)
        es.close()
    nc.compile()
    return nc


def _host_prep(inputs):
    hs = np.ascontiguousarray(np.asarray(inputs["hidden_states"], dtype=np.float32))
    cos = np.asarray(inputs["cos"], dtype=np.float32)
    sin = np.asarray(inputs["sin"], dtype=np.float32)
    wq = np.asarray(inputs["wq"], dtype=np.float32)
    wk = np.asarray(inputs["wk"], dtype=np.float32)
    wv = np.asarray(inputs["wv"], dtype=np.float32)
    wo = np.asarray(inputs["wo"], dtype=np.float32)

    cosT = np.ascontiguousarray(cos.T)
    sin2 = np.concatenate([sin[:, D // 2:], sin[:, :D // 2]], axis=1)
    sin2T = np.ascontiguousarray(sin2.T)

    rot = np.zeros((D, D), dtype=np.float32)
    half = D // 2
    for d in range(half):
        rot[d, d + half] = -1.0
    for d in range(half, D):
        rot[d, d - half] = 1.0
    rotT = np.ascontiguousarray(rot.T)

    # multiplicative post-exp masks per relative k-tile offset
    maskb = np.zeros((4, P, QTW), dtype=np.float32)
    i = np.arange(P)[:, None]
    j = np.arange(QTW)[None, :]
    for bi, t in enumerate((0, 1, 4, 5)):
        delta = -WIN + t * P
        maskb[bi] = np.where(np.abs(delta + i - j) <= WIN, 0.0, -60000.0)
    maskb = np.tile(maskb, (1, 1, 2))  # duplicated for the 2-head pairing

    cdt = mybir.dt.np(CDT)
    mdt = mybir.dt.np(MDT)
    hsT = [np.ascontiguousarray(hs[b].T).astype(cdt) for b in range(B)]
    in_maps = []
    for c in range(N_CORES):
        b, g = divmod(c, NKV)
        in_maps.append({
            "hsT": hsT[b],
            "wq_t": np.ascontiguousarray(wq[g * DQ:(g + 1) * DQ, :].T).astype(cdt),
            "wk_t": np.ascontiguousarray(wk[g * D:(g + 1) * D, :].T).astype(cdt),
            "wv_t": np.ascontiguousarray(wv[g * D:(g + 1) * D, :].T).astype(cdt),
            "wo_t": np.ascontiguousarray(wo[:, g * DQ:(g + 1) * DQ].T).astype(cdt),
            "cos_t": cosT,
            "sin2_t": sin2T,
            "rot_t": rotT.astype(cdt),
            "ones_d": np.ones((P, P), dtype=cdt),
            "maskb": maskb.astype(cdt),
            "ident_d": np.eye(P, dtype=np.float32).astype(cdt),
        })
    return in_maps


def kernel(**inputs):
    from concourse.bass_utils import run_bass_kernel_spmd
    if "nc" not in _CACHE:
        _CACHE["nc"] = build_nc()
    nc = _CACHE["nc"]
    in_maps = _host_prep(inputs)
    trace = bool(int(os.environ.get("BASS_TRACE_RUN", "0")))
    kw = {}
    td = os.environ.get("BASS_TRACE_DIR")
    if td:
        os.makedirs(td, exist_ok=True)
        kw["tmpdir"] = td
    res = run_bass_kernel_spmd(nc, in_maps, core_ids=list(range(N_CORES)), trace=trace, **kw)
    _CACHE["last_results"] = res
    out = np.empty((B, S, NHQ * D), dtype=np.float32)
    for b in range(B):
        acc = res.results[4 * b]["outT"].astype(np.float32, copy=True)
        for g in range(1, NKV):
            acc += res.results[4 * b + g]["outT"]
        out[b] = acc.T
    return out


if __name__ == "__main__":
    nc = build_nc()
    print("built OK; instructions:",
          sum(1 for _ in nc.m.functions[0].instructions)
          if hasattr(nc.m.functions[0], "instructions") else "?")


# revision 24
# speedup vs baseline: 1.0336x; 1.0336x over previous
"""Trainium2 Bass kernel for AceStep sliding-window GQA attention.

Problem: B=2, S=2048, H=2048, 16 Q heads / 4 KV heads, D=128, window +-256, fp32.

Sharding: 8 cores = (batch b in {0,1}) x (kv-group g in {0..3}).
Each core owns 4 Q heads + 1 KV head and computes a partial output
(wo restricted to its head group); host sums 4 partials per batch.

On-device layout is fully transposed ([dim, token]) so that:
  - QKV projections:  qT[d,s] = wqT[H,d].T @ hsT[H,s]          (PE matmul)
  - RoPE rotate_half: rot(q) = R @ q  (128x128 rotation matrix) (PE matmul)
  - RMSNorm sum over d and softmax denominator sum over k
    (partition-axis reductions) via ones-vector matmuls          (PE matmul)
  - scoresT[k,q] = kT[d,k].T @ qT[d,q]                          (PE matmul)
  - PV: outT[d,q] = v_kd[k,d].T @ probsT[k,q]                   (PE matmul)
  - O-proj: finalT[ho,s] = woT[dq,ho].T @ attnT[dq,s]           (PE matmul)
Softmax is computed without max-subtraction: RMS-normed q,k bound
|score| <= sqrt(128) ~ 11.3, so exp stays in fp32 range.
Sliding window exploited at block level: only ~6 of 16 k-tiles per q-tile.
Matmuls run as float32r (full PE rate at N>=256, near-fp32 precision).
"""

import os
import sys
from contextlib import ExitStack

import numpy as np

for _p in ("/opt/trn_rl_repo", "/root/.axon_site/_ro/trn_rl_repo"):
    if os.path.isdir(_p) and _p not in sys.path:
        sys.path.insert(0, _p)

import concourse.bass as bass
import concourse.bacc as bacc
import concourse.mybir as mybir
from concourse import tile
from concourse.alu_op_type import AluOpType
from concourse import bass_isa

F32 = mybir.dt.float32
F32R = mybir.dt.float32r
BF16 = mybir.dt.bfloat16
CDT = BF16 if os.environ.get("BASS_CDT", "f32r") == "bf16" else F32R
MDT = BF16 if CDT == BF16 else F32   # mask dtype
ACT = mybir.ActivationFunctionType

# problem dims (hardcoded per spec)
B, S, H, NHQ, NKV, D, WIN = 2, 2048, 2048, 16, 4, 128, 256
EPS = 1e-6
HPC = NHQ // NKV          # 4 q heads per core
DQ = HPC * D              # 512
P = 128
KT = H // P               # 16 contraction tiles
SQ = 512                  # s-quarter width for projections
NSQ = S // SQ
QTW = 256                 # attention q-tile width
NQT = S // QTW
NKTILES = (QTW + 2 * WIN) // P   # 6 k-tiles per q-tile
N_CORES = 8

_CACHE = {}


def build_nc():
    nc = bacc.Bacc(None, target_bir_lowering=False, debug=False)

    hsT = nc.dram_tensor("hsT", [H, S], CDT, kind="ExternalInput")
    wq_t = nc.dram_tensor("wq_t", [H, DQ], CDT, kind="ExternalInput")
    wk_t = nc.dram_tensor("wk_t", [H, D], CDT, kind="ExternalInput")
    wv_t = nc.dram_tensor("wv_t", [H, D], CDT, kind="ExternalInput")
    wo_t = nc.dram_tensor("wo_t", [DQ, H], CDT, kind="ExternalInput")
    cos_t = nc.dram_tensor("cos_t", [D, S], F32, kind="ExternalInput")
    sin2_t = nc.dram_tensor("sin2_t", [D, S], F32, kind="ExternalInput")
    rot_t = nc.dram_tensor("rot_t", [D, D], CDT, kind="ExternalInput")
    ones_d = nc.dram_tensor("ones_d", [P, P], CDT, kind="ExternalInput")
    maskb_d = nc.dram_tensor("maskb", [4, P, 2 * QTW], CDT, kind="ExternalInput")
    ident_d = nc.dram_tensor("ident_d", [P, P], CDT, kind="ExternalInput")
    outT = nc.dram_tensor("outT", [H, S], F32, kind="ExternalOutput")

    with tile.TileContext(nc) as tc:
        es = ExitStack()
        top = es.enter_context(tc.tile_pool(name="top", bufs=1))

        # const APs used by nc.scalar.activation float biases
        zc = top.tile([P, 1], F32)
        nc.vector.memset(zc[:, :], 0.0)
        nc.const_aps.aps[(F32, 0.0)] = zc[:, :]
        bq = top.tile([P, 1], F32)
        nc.vector.memset(bq[:, :], float(D * EPS))
        nc.const_aps.aps[(F32, float(D * EPS))] = bq[:, :]
        bk = top.tile([P, 1], F32)
        nc.vector.memset(bk[:, :], float(EPS))
        nc.const_aps.aps[(F32, float(EPS))] = bk[:, :]

        ident = top.tile([P, P], F32)
        nc.vector.memset(ident[:, :], 1.0)
        nc.gpsimd.affine_select(
            out=ident[:, :], in_=ident[:, :], pattern=[[-1, P]],
            compare_op=AluOpType.is_equal, fill=0.0, base=0, channel_multiplier=1,
        )
        rot_sb = top.tile([D, D], CDT)
        nc.sync.dma_start(out=rot_sb[:, :], in_=rot_t[:, :])
        ones_t = top.tile([P, P], CDT)
        nc.sync.dma_start(out=ones_t[:, :], in_=ones_d[:, :])

        qTall = top.tile([P, HPC * S], CDT, name="qTall")  # col = qi*1024 + h*256 + q
        kTt = top.tile([P, S], CDT, name="kTt")
        vkd = top.tile([P, S], CDT, name="vkd")  # s-tile t at [:, t*P:(t+1)*P], [s,d] layout
        attnT = [top.tile([P, S], CDT, name=f"attnT{h}") for h in range(HPC)]

        # ---------------- Phase 1: QKV projections + RMSNorm + RoPE ----------
        with tc.tile_pool(name="ph1", bufs=1) as ph1, \
             tc.tile_pool(name="ph1p", bufs=1, space="PSUM") as ph1p:
            wq_sb = ph1.tile([P, KT * DQ], CDT)
            wk_sb = ph1.tile([P, KT * D], CDT)
            wv_sb = ph1.tile([P, KT * D], CDT)
            cos_sb = ph1.tile([D, S], F32)
            sin2_sb = ph1.tile([D, S], F32)

            def load_weights_k(k):
                nc.sync.dma_start(out=wq_sb[:, k * DQ:(k + 1) * DQ], in_=wq_t[k * P:(k + 1) * P, :])
                nc.sync.dma_start(out=wk_sb[:, k * D:(k + 1) * D], in_=wk_t[k * P:(k + 1) * P, :])
                nc.sync.dma_start(out=wv_sb[:, k * D:(k + 1) * D], in_=wv_t[k * P:(k + 1) * P, :])

            for sq in range(NSQ):
                s0 = sq * SQ
                hst = []
                for k in range(KT):
                    if sq == 0:
                        load_weights_k(k)  # interleave so matmuls start early
                    t = ph1.tile([P, SQ], CDT, tag="hst", bufs=8)
                    nc.sync.dma_start(out=t[:, :], in_=hsT[k * P:(k + 1) * P, s0:s0 + SQ])
                    hst.append(t)
                if sq == 0:
                    nc.sync.dma_start(out=cos_sb[:, :], in_=cos_t[:, :])
                    nc.sync.dma_start(out=sin2_sb[:, :], in_=sin2_t[:, :])

                accs = [ph1p.tile([P, SQ], F32, tag=f"acc{m}", bufs=1, name=f"acc{m}_{sq}")
                        for m in range(HPC + 2)]
                for k in range(KT):
                    st, sp = (k == 0), (k == KT - 1)
                    for m in range(HPC):
                        nc.tensor.matmul(
                            accs[m][:, :],
                            wq_sb[:, k * DQ + m * D: k * DQ + (m + 1) * D],
                            hst[k][:, :], start=st, stop=sp)
                    nc.tensor.matmul(accs[HPC][:, :], wk_sb[:, k * D:(k + 1) * D],
                                     hst[k][:, :], start=st, stop=sp)
                    nc.tensor.matmul(accs[HPC + 1][:, :], wv_sb[:, k * D:(k + 1) * D],
                                     hst[k][:, :], start=st, stop=sp)

                # v first: releases its accumulator early
                vsb = ph1.tile([P, SQ], F32, tag="tmp", bufs=6, name=f"vsb{sq}")
                nc.scalar.copy(vsb[:, :], accs[HPC + 1][:, :])
                for j in range(SQ // P):
                    vt = ph1p.tile([P, P], F32, tag="rot", bufs=2, name=f"vt{sq}_{j}")
                    nc.tensor.transpose(vt[:, :], vsb[:, j * P:(j + 1) * P], ident[:, :])
                    nc.vector.tensor_copy(vkd[:, s0 + j * P: s0 + (j + 1) * P], vt[:, :])

                # q heads + k: RMSNorm (scale folded for q) + RoPE.
                # Stage-batched per engine so the slow gpsimd partition reduces
                # never head-of-line-block the ACT/DVE FIFOs that release the
                # PSUM accumulators for the next quarter.
                sqts, t1s, t2s, rots, ssqBs, rmsBs, invBs_l = {}, {}, {}, {}, {}, {}, {}
                for m in range(HPC + 1):
                    raw = accs[m]
                    sqt = ph1.tile([P, SQ], CDT, tag="sqt", bufs=4, name=f"sqt{sq}_{m}")
                    nc.scalar.activation(sqt[:, :], raw[:, :], ACT.Square)
                    sqts[m] = sqt
                    t1 = ph1.tile([P, SQ], CDT, tag="t1", bufs=4, name=f"t1_{sq}_{m}")
                    nc.vector.tensor_mul(t1[:, :], raw[:, :], sin2_sb[:, s0:s0 + SQ])
                    t1s[m] = t1
                    t2 = ph1.tile([P, SQ], F32, tag="tmp", bufs=6, name=f"t2_{sq}_{m}")
                    nc.vector.tensor_mul(t2[:, :], raw[:, :], cos_sb[:, s0:s0 + SQ])
                    t2s[m] = t2
                for m in range(HPC + 1):
                    rotp = ph1p.tile([P, SQ], F32, tag="rot", bufs=2, name=f"rot{sq}_{m}")
                    nc.tensor.matmul(rotp[:, :], rot_sb[:, :], t1s[m][:, :])
                    rots[m] = rotp
                    ssqB = ph1.tile([P, SQ], F32, tag="ssqB", bufs=2, name=f"ssqB{sq}_{m}")
                    nc.gpsimd.partition_all_reduce(ssqB[:, :], sqts[m][:, :], channels=P,
                                                   reduce_op=bass_isa.ReduceOp.add)
                    ssqBs[m] = ssqB
                for m in range(HPC + 1):
                    rmsB = ph1.tile([P, SQ], F32, tag="rmsB", bufs=2, name=f"rmsB{sq}_{m}")
                    if m < HPC:
                        nc.scalar.activation(rmsB[:, :], ssqBs[m][:, :], ACT.Sqrt,
                                             bias=float(D * EPS), scale=1.0)
                    else:
                        nc.scalar.activation(rmsB[:, :], ssqBs[m][:, :], ACT.Sqrt,
                                             bias=float(EPS), scale=1.0 / D)
                    rmsBs[m] = rmsB
                    # rope combine can run as soon as rotp lands
                    t3 = ph1.tile([P, SQ], F32, tag="t3", bufs=3, name=f"t3_{sq}_{m}")
                    nc.vector.tensor_add(t3[:, :], t2s[m][:, :], rots[m][:, :])
                    t2s[m] = t3
                for m in range(HPC + 1):
                    invB = ph1.tile([P, SQ], F32, tag="invBs", bufs=2, name=f"invB{sq}_{m}")
                    nc.vector.reciprocal_approx_fast(out=invB[:, :], in_=rmsBs[m][:, :])
                    if m < HPC:
                        # [128, 2, 256] strided view: quarter sq covers qi = 2sq, 2sq+1
                        dst = qTall[:, :].rearrange(
                            "p (qi h q) -> p qi h q", h=HPC, q=QTW)[:, 2 * sq:2 * sq + 2, m, :]
                        nc.vector.tensor_mul(dst, t2s[m][:, :].rearrange("p (a q) -> p a q", a=2),
                                             invB[:, :].rearrange("p (a q) -> p a q", a=2))
                    else:
                        nc.vector.tensor_mul(kTt[:, s0:s0 + SQ], t2s[m][:, :], invB[:, :])

        # ---------------- Phase 2+3 -----------------------------------------
        with tc.tile_pool(name="late", bufs=1) as late:
            wo_sb = late.tile([P, HPC * H], CDT)
            for dqt in range(HPC):
                nc.sync.dma_start(out=wo_sb[:, dqt * H:(dqt + 1) * H],
                                  in_=wo_t[dqt * P:(dqt + 1) * P, :])

            # ------- attention (qi-outer, heads inner) + interleaved O-proj ----
            with tc.tile_pool(name="att", bufs=1) as att, \
                 tc.tile_pool(name="attp", bufs=1, space="PSUM") as attp:
                maskb_sb = att.tile([P, 4 * 2 * QTW], CDT)
                for t in range(4):
                    nc.sync.dma_start(out=maskb_sb[:, t * 2 * QTW:(t + 1) * 2 * QTW], in_=maskb_d[t])
                identr_sb = att.tile([P, P], CDT)
                nc.sync.dma_start(out=identr_sb[:, :], in_=ident_d[:, :])

                def attend_pair(hp, qi):
                    q0 = qi * QTW
                    col0 = qi * (HPC * QTW) + hp * (2 * QTW)
                    W2 = 2 * QTW
                    tl = [t for t in range(NKTILES) if 0 <= q0 - WIN + t * P <= S - P]
                    L = len(tl)
                    probs_all = att.tile([P, NKTILES * W2], CDT, tag="probs", bufs=3,
                                         name=f"probs{hp}_{qi}")
                    BIDX = {0: 0, 1: 1, 4: 2, 5: 3}
                    for t in tl:
                        ks = q0 - WIN + t * P
                        scp = attp.tile([P, W2], F32, tag="sc", bufs=2, name=f"sc{hp}_{qi}_{t}")
                        masked = t in BIDX
                        nc.tensor.matmul(scp[:, :], kTt[:, ks:ks + P],
                                         qTall[:, col0:col0 + W2],
                                         start=True, stop=not masked)
                        if masked:  # window mask as additive bias accumulated on PE
                            bi = BIDX[t]
                            nc.tensor.matmul(scp[:, :], identr_sb[:, :],
                                             maskb_sb[:, bi * W2:(bi + 1) * W2],
                                             start=False, stop=True)
                        nc.scalar.activation(probs_all[:, t * W2:(t + 1) * W2],
                                             scp[:, :], ACT.Exp)
                    den = attp.tile([1, W2], F32, tag="den", bufs=1, name=f"den{hp}_{qi}")
                    pvs2 = []
                    for h2 in range(2):
                        pv = attp.tile([P, QTW], F32, tag="pv", bufs=2, name=f"pv{hp}_{qi}_{h2}")
                        for i, t in enumerate(tl):
                            ks = q0 - WIN + t * P
                            nc.tensor.matmul(pv[:, :], vkd[:, ks:ks + P],
                                             probs_all[:, t * W2 + h2 * QTW: t * W2 + (h2 + 1) * QTW],
                                             start=(i == 0), stop=(i == L - 1))
                        pvs = att.tile([P, QTW], F32, tag="pvs", bufs=4, name=f"pvs{hp}_{qi}_{h2}")
                        nc.vector.tensor_copy(pvs[:, :], pv[:, :])
                        pvs2.append(pvs)
                    for i, t in enumerate(tl):
                        nc.tensor.matmul(den[:, :], ones_t[:, 0:1],
                                         probs_all[:, t * W2:(t + 1) * W2],
                                         start=(i == 0), stop=(i == L - 1))
                    invf = att.tile([1, W2], F32, tag="invf", bufs=3, name=f"invf{hp}_{qi}")
                    nc.vector.reciprocal_approx_fast(out=invf[:, :], in_=den[:, :])
                    invr = att.tile([1, W2], CDT, tag="invr", bufs=3, name=f"invr{hp}_{qi}")
                    nc.scalar.copy(invr[:, :], invf[:, :])
                    invB = attp.tile([P, W2], F32, tag="ainvB", bufs=1, name=f"ainvB{hp}_{qi}")
                    nc.tensor.matmul(invB[:, :], ones_t[0:1, :], invr[:, :])
                    for h2 in range(2):
                        nc.vector.tensor_mul(attnT[2 * hp + h2][:, q0:q0 + QTW],
                                             pvs2[h2][:, :],
                                             invB[:, h2 * QTW:(h2 + 1) * QTW])

                def oproj_block(st4):
                    s0 = st4 * SQ
                    for ho in range(H // P):
                        ops = attp.tile([P, SQ], F32, tag="o", bufs=2, name=f"o{st4}_{ho}")
                        for dqt in range(HPC):
                            nc.tensor.matmul(
                                ops[:, :],
                                wo_sb[:, dqt * H + ho * P: dqt * H + (ho + 1) * P],
                                attnT[dqt][:, s0:s0 + SQ],
                                start=(dqt == 0), stop=(dqt == HPC - 1))
                        ob = att.tile([P, SQ], F32, tag="ob", bufs=4, name=f"ob{st4}_{ho}")
                        if (ho + st4) % 2 == 0:
                            nc.scalar.copy(ob[:, :], ops[:, :])
                        else:
                            nc.vector.tensor_copy(ob[:, :], ops[:, :])
                        nc.sync.dma_start(out=outT[ho * P:(ho + 1) * P, s0:s0 + SQ], in_=ob[:, :])

                for qi in range(NQT):
                    for hp in range(HPC // 2):
                        attend_pair(hp, qi)
                    if qi % 2 == 1:
                        oproj_block(qi // 2)
        es.close()
    nc.compile()
    return nc


def _host_prep(inputs):
    hs = np.ascontiguousarray(np.asarray(inputs["hidden_states"], dtype=np.float32))
    cos = np.asarray(inputs["cos"], dtype=np.float32)
    sin = np.asarray(inputs["sin"], dtype=np.float32)
    wq = np.asarray(inputs["wq"], dtype=np.float32)
    wk = np.asarray(inputs["wk"], dtype=np.float32)
    wv = np.asarray(inputs["wv"], dtype=np.float32)
    wo = np.asarray(inputs["wo"], dtype=np.float32)

    cosT = np.ascontiguousarray(cos.T)
    sin2 = np.concatenate([sin[:, D // 2:], sin[:, :D // 2]], axis=1)
    sin2T = np.ascontiguousarray(sin2.T)

    rot = np.zeros((D, D), dtype=np.float32)
    half = D // 2
    for d in range(half):
        rot[d, d + half] = -1.0
    for d in range(half, D):
        rot[d, d - half] = 1.0
    rotT = np.ascontiguousarray(rot.T)

    # multiplicative post-exp masks per relative k-tile offset
    maskb = np.zeros((4, P, QTW), dtype=np.float32)
    i = np.arange(P)[:, None]
    j = np.arange(QTW)[None, :]
    for bi, t in enumerate((0, 1, 4, 5)):
        delta = -WIN + t * P
        maskb[bi] = np.where(np.abs(delta + i - j) <= WIN, 0.0, -60000.0)
    maskb = np.tile(maskb, (1, 1, 2))  # duplicated for the 2-head pairing

    cdt = mybir.dt.np(CDT)
    mdt = mybir.dt.np(MDT)
    hsT = [np.ascontiguousarray(hs[b].T).astype(cdt) for b in range(B)]
    in_maps = []
    for c in range(N_CORES):
        b, g = divmod(c, NKV)
        in_maps.append({
            "hsT": hsT[b],
            "wq_t": np.ascontiguousarray(wq[g * DQ:(g + 1) * DQ, :].T).astype(cdt),
            "wk_t": np.ascontiguousarray(wk[g * D:(g + 1) * D, :].T).astype(cdt),
            "wv_t": np.ascontiguousarray(wv[g * D:(g + 1) * D, :].T).astype(cdt),
            "wo_t": np.ascontiguousarray(wo[:, g * DQ:(g + 1) * DQ].T).astype(cdt),
            "cos_t": cosT,
            "sin2_t": sin2T,
            "rot_t": rotT.astype(cdt),
            "ones_d": np.ones((P, P), dtype=cdt),
            "maskb": maskb.astype(cdt),
            "ident_d": np.eye(P, dtype=np.float32).astype(cdt),
        })
    return in_maps


def kernel(**inputs):
    from concourse.bass_utils import run_bass_kernel_spmd
    if "nc" not in _CACHE:
        _CACHE["nc"] = build_nc()
    nc = _CACHE["nc"]
    in_maps = _host_prep(inputs)
    trace = bool(int(os.environ.get("BASS_TRACE_RUN", "0")))
    kw = {}
    td = os.environ.get("BASS_TRACE_DIR")
    if td:
        os.makedirs(td, exist_ok=True)
        kw["tmpdir"] = td
    res = run_bass_kernel_spmd(nc, in_maps, core_ids=list(range(N_CORES)), trace=trace, **kw)
    _CACHE["last_results"] = res
    out = np.empty((B, S, NHQ * D), dtype=np.float32)
    for b in range(B):
        acc = res.results[4 * b]["outT"].astype(np.float32, copy=True)
        for g in range(1, NKV):
            acc += res.results[4 * b + g]["outT"]
        out[b] = acc.T
    return out


if __name__ == "__main__":
    nc = build_nc()
    print("built OK; instructions:",
          sum(1 for _ in nc.m.functions[0].instructions)
          if hasattr(nc.m.functions[0], "instructions") else "?")
